# revision 1
# baseline (speedup 1.0000x reference)
"""AWBNet (wo R2) Trainium2 kernel.

Math (per sample b):
  m = reshape(relu(hist_flat @ W1 + b1) @ W2 + b2, [9, 3])
  feats(px) = [r, g, b, r^2, g^2, b^2, rg, rb, gb]
  y[px, c] = sum_k feats[px, k] * m[k, c]

Device strategy (8 cores, pure data parallel over batch, 2 samples/core):
  * Tiny MLP on TensorE in fp32 with natural layouts (host only re-packs
    histogram / b1 so no on-device transposes are needed).
  * Per-pixel einsum on VectorE/ScalarE in fp16 using the square basis
    {R, G, B, R^2, G^2, B^2, (R+G)^2, (R+B)^2, (G+B)^2}; the coefficient
    change (rg = ((R+G)^2 - R^2 - G^2)/2 etc.) is folded into W2/b2 on the
    host (pure linear re-parameterization of the weights, no data compute).
  * m-coefficients are broadcast to all 128 partitions by a fused
    matmul whose lhsT is a stride-0 (broadcast) column of featT; they are
    then per-partition scalars for the per-pixel products.
  * Per-pixel: ScalarE deinterleaves (stride-3 fp32 -> dense fp16) and
    squares; products m_k*F_k split DVE tensor_scalar (2x) / ACT
    scale-copies to balance the engines; DVE tt-add trees combine, the
    last add writing the stride-3 fp32 output view directly.
  * Three DMA queues in parallel: W1 stream + late x tiles on SWDGE
    (with fp32->fp16 cast), x0/x1 + y stores on the SP HWDGE ring, small
    setup DMAs on the ACT HWDGE ring.
"""

import sys

import numpy as np

for _p in ("/opt/trn_rl_repo",):
    if _p not in sys.path:
        sys.path.insert(0, _p)

import concourse.bacc as bacc
import concourse.mybir as mybir
import concourse.tile as tile
from concourse import bass_utils

# ---- problem constants (hardcoded per contract) ----
N_CORES = 8
B, H, W, C = 16, 512, 512, 3
SPC = B // N_CORES  # samples per core = 2
PX_SAMPLE = H * W  # 262144
PX_CORE = SPC * PX_SAMPLE  # 524288
P = 128
LANE_PX = PX_CORE // P  # 4096 pixels per partition per core
T = 1024  # pixels per partition per tile
NTILES = LANE_PX // T  # 4
TILES_PER_SAMPLE = NTILES // SPC  # 2

HIST = 3 * 64 * 64  # 12288
HID = 256
MOUT = 27
KT = HIST // P  # 96 k-tiles
MT = HID // P  # 2 m-tiles
W1_CH = 8  # k-tiles per W1 DMA chunk (8 * 128KB = 1MB)
KT_SH = KT // N_CORES  # 12 k-tiles of W1 per core (K-sharded MLP + AllReduce)

F16 = mybir.dt.float16
BF16 = mybir.dt.bfloat16
PLDT = mybir.dt.float16
F32 = mybir.dt.float32
MULT = mybir.AluOpType.mult
ADD = mybir.AluOpType.add
AF = mybir.ActivationFunctionType

_CACHE = {}


def _coeff_transform():
    """T27 so that m' = m_flat @ T27.T gives coefficients for the square
    basis [R,G,B,R2,G2,B2,(R+G)^2,(R+B)^2,(G+B)^2]."""
    T9 = np.zeros((9, 9), dtype=np.float64)
    for i in range(3):  # R,G,B linear terms pass through
        T9[i, i] = 1.0
    # new squares: old squares minus half the relevant cross terms
    # old order: 3=r2,4=g2,5=b2,6=rg,7=rb,8=gb
    T9[3, 3] = 1.0
    T9[3, 6] = -0.5
    T9[3, 7] = -0.5
    T9[4, 4] = 1.0
    T9[4, 6] = -0.5
    T9[4, 8] = -0.5
    T9[5, 5] = 1.0
    T9[5, 7] = -0.5
    T9[5, 8] = -0.5
    T9[6, 6] = 0.5  # (R+G)^2 coeff = rg/2
    T9[7, 7] = 0.5
    T9[8, 8] = 0.5
    T27 = np.zeros((27, 27), dtype=np.float64)
    for c in range(3):
        for kn in range(9):
            for ko in range(9):
                T27[3 * kn + c, 3 * ko + c] = T9[kn, ko]
    return T27


def _build():
    nc = bacc.Bacc(
        "TRN2", target_bir_lowering=False, debug=False, num_devices=N_CORES
    )

    x_d = nc.dram_tensor("x_core", [NTILES, P, T * C], F32, kind="ExternalInput")
    hp_d = nc.dram_tensor("h_packed", [P, KT * SPC], F32, kind="ExternalInput")
    w1_d = nc.dram_tensor("w1", [KT, P, HID], F32, kind="ExternalInput")
    b1_d = nc.dram_tensor("b1_rep", [SPC, HID], F32, kind="ExternalInput")
    w2_d = nc.dram_tensor("w2p", [MT, P, MOUT], F32, kind="ExternalInput")
    b2_d = nc.dram_tensor("b2bc", [P, SPC * MOUT], F32, kind="ExternalInput")
    eye_d = nc.dram_tensor("eye2", [SPC, SPC], F32, kind="ExternalInput")
    y_d = nc.dram_tensor("y_core", [NTILES, P, T * C], F32, kind="ExternalOutput")

    with tile.TileContext(nc) as tc:
        with (
            tc.tile_pool(name="mlp", bufs=1) as mlp_pool,
            tc.tile_pool(name="w1s", bufs=3) as w1_pool,
            tc.tile_pool(name="px32", bufs=2) as px32_pool,
            tc.tile_pool(name="pl16", bufs=2) as plane_pool,
            tc.tile_pool(name="ps", bufs=1, space="PSUM") as psum_pool,
        ):
            # ---------------- MLP (TensorE) ----------------
            hp_sb = mlp_pool.tile([P, KT * SPC], F16, tag="hp", name="hp")
            nc.gpsimd.dma_start(out=hp_sb, in_=hp_d[:, :])
            b1_sb = mlp_pool.tile([SPC, HID], F32, tag="b1", name="b1")
            nc.scalar.dma_start(out=b1_sb, in_=b1_d[:, :])
            w2_sb = mlp_pool.tile([P, MT, MOUT], F32, tag="w2", name="w2")
            nc.scalar.dma_start(out=w2_sb, in_=w2_d.rearrange("m p n -> p m n"))
            b2_sb = mlp_pool.tile([P, SPC * MOUT], F32, tag="b2", name="b2")
            nc.scalar.dma_start(out=b2_sb, in_=b2_d[:, :])
            eye_sb = mlp_pool.tile([SPC, SPC], F32, tag="eye", name="eye")
            nc.scalar.dma_start(out=eye_sb, in_=eye_d[:, :])

            # feat = h @ W1: lhsT = h-slices [128, 2] (cheap weight loads),
            # rhs = W1 k-tiles [128, 256] -> psum [2, 256] accumulated.
            feat_ps = psum_pool.tile([SPC, HID], F32, tag="featps", name="featps")
            for kc in range(KT // W1_CH):
                w1_sb = w1_pool.tile([P, W1_CH, HID], F16, tag="w1c", name="w1c")
                nc.gpsimd.dma_start(
                    out=w1_sb,
                    in_=w1_d[kc * W1_CH : (kc + 1) * W1_CH].rearrange(
                        "k p n -> p k n"
                    ),
                )
                for kk in range(W1_CH):
                    k = kc * W1_CH + kk
                    nc.tensor.matmul(
                        feat_ps,
                        hp_sb[:, k * SPC : (k + 1) * SPC],
                        w1_sb[:, kk, :],
                        start=(k == 0),
                        stop=(k == KT - 1),
                    )

            # relu(feat + b1) on DVE (b1 lives on the free dim here)
            feat_sb = mlp_pool.tile([SPC, HID], F32, tag="featsb", name="featsb")
            nc.vector.tensor_add(feat_sb, feat_ps, b1_sb)
            feat_r = mlp_pool.tile([SPC, HID], F32, tag="featr", name="featr")
            nc.vector.tensor_scalar(
                feat_r, feat_sb, 0.0, None, mybir.AluOpType.max
            )

            # transpose feat [2, 256] -> featT tiles [128, 2] via PE
            featT_sb = []
            for mt in range(MT):
                ft_ps = psum_pool.tile(
                    [P, SPC], F32, tag=f"ftps{mt}", name=f"ftps{mt}"
                )
                nc.tensor.transpose(
                    ft_ps, feat_r[:, mt * P : (mt + 1) * P], eye_sb
                )
                ft_sb = mlp_pool.tile(
                    [P, SPC], F32, tag=f"ftsb{mt}", name=f"ftsb{mt}"
                )
                nc.vector.tensor_copy(ft_sb, ft_ps)
                featT_sb.append(ft_sb)

            # fused m-matmul + partition-broadcast: a stride-0 lhsT column
            # makes every output partition compute m[s] = featT[:, s] @ W2'.
            mb_ps = psum_pool.tile([P, SPC * MOUT], F32, tag="mbps", name="mbps")
            for s in range(SPC):
                for mt in range(MT):
                    nc.tensor.matmul(
                        mb_ps[:, s * MOUT : (s + 1) * MOUT],
                        featT_sb[mt][:, s : s + 1].broadcast_to([P, P]),
                        w2_sb[:, mt, :],
                        start=(mt == 0),
                        stop=(mt == MT - 1),
                    )
            mscal = mlp_pool.tile([P, SPC * MOUT], F32, tag="mscal", name="mscal")
            nc.vector.tensor_add(mscal, mb_ps, b2_sb)

            # ---------------- pixel path ----------------
            for t in range(NTILES):
                s = t // TILES_PER_SAMPLE

                def ms(k, c, s=s):
                    j = s * MOUT + 3 * k + c
                    return mscal[:, j : j + 1]

                x32 = px32_pool.tile([P, T, C], F32, tag="x32", name="x32")
                x_dma = nc.sync if t < 2 else nc.gpsimd
                x_dma.dma_start(out=x32, in_=x_d[t].rearrange("p (t c) -> p t c", c=C))

                # deinterleave + cast to fp16 into channel-slices of one
                # wide [P, 3, T] tile (ACT, stride-3 reads)
                rgb = plane_pool.tile([P, C, T], PLDT, tag="rgb", name="rgb")
                nc.scalar.copy(rgb, x32.rearrange("p t c -> p c t"))

                # pair sums (DVE fp16 2x) into a wide tile
                sm = plane_pool.tile([P, C, T], PLDT, tag="sm", name="sm")
                nc.vector.tensor_add(sm[:, 0, :], rgb[:, 0, :], rgb[:, 1, :])
                nc.vector.tensor_add(sm[:, 1, :], rgb[:, 0, :], rgb[:, 2, :])
                nc.vector.tensor_add(sm[:, 2, :], rgb[:, 1, :], rgb[:, 2, :])

                # squares: two wide ACT ops cover all six planes
                sq = plane_pool.tile([P, C, T], PLDT, tag="sq", name="sq")
                qq = plane_pool.tile([P, C, T], PLDT, tag="qq", name="qq")
                nc.scalar.square(sq, rgb)
                nc.scalar.square(qq, sm)

                basis = [
                    rgb[:, 0, :], rgb[:, 1, :], rgb[:, 2, :],
                    sq[:, 0, :], sq[:, 1, :], sq[:, 2, :],
                    qq[:, 0, :], qq[:, 1, :], qq[:, 2, :],
                ]

                y32 = px32_pool.tile([P, T, C], F32, tag="y32", name="y32")
                y32r = y32.rearrange("p t c -> p c t")
                # products: per-channel (distinct scalars) into channel-slices
                # of wide U tiles; adds: channel-merged [P, 3, T] tree.
                # products on ScalarE: 4 per channel for the first half of
                # the tiles, 3 for the rest (balances ACT vs DVE busy time)
                ACT_K = (3, 4, 5, 6) if t < 2 else (3, 4, 5)

                def prods(k, uname):
                    uk = plane_pool.tile(
                        [P, C, T], PLDT, tag=uname, name=f"{uname}_{k}"
                    )
                    for c in range(C):
                        if k in ACT_K:
                            nc.scalar.mul(uk[:, c, :], basis[k], ms(k, c))
                        else:
                            nc.vector.tensor_scalar(
                                uk[:, c, :], basis[k], ms(k, c), None, MULT
                            )
                    return uk

                def tadd(tag, nm, a, b_):
                    o = plane_pool.tile([P, C, T], PLDT, tag=tag, name=nm)
                    nc.vector.tensor_add(o, a, b_)
                    return o

                ua = prods(0, "ua")
                ub = prods(1, "ub")
                ta1 = tadd("ta", f"ta1_{t}", ua, ub)
                ua = prods(2, "ua")
                ub = prods(3, "ub")
                tb1 = tadd("tb", f"tb1_{t}", ua, ub)
                tc1 = tadd("tc", f"tc1_{t}", ta1, tb1)
                ua = prods(4, "ua")
                ub = prods(5, "ub")
                ta2 = tadd("ta", f"ta2_{t}", ua, ub)
                ua = prods(6, "ua")
                ub = prods(7, "ub")
                tb2 = tadd("tb", f"tb2_{t}", ua, ub)
                ta3 = tadd("ta", f"ta3_{t}", ta2, tb2)
                ua = prods(8, "ua")
                tc2 = tadd("tc", f"tc2_{t}", tc1, ua)
                nc.vector.tensor_add(y32r, ta3, tc2)

                nc.sync.dma_start(
                    out=y_d[t].rearrange("p (t c) -> p t c", c=C), in_=y32
                )

    nc.compile()
    return nc


def _prep_inputs(x, histogram, W1, b1, W2, b2):
    """Host-side sharding / layout packing (no arithmetic on data except the
    static linear re-parameterization of the tiny weights W2/b2)."""
    x = np.ascontiguousarray(np.asarray(x, dtype=np.float32))
    hist = np.asarray(histogram, dtype=np.float32).reshape(B, HIST)
    W1 = np.ascontiguousarray(np.asarray(W1, dtype=np.float32))
    b1 = np.asarray(b1, dtype=np.float32)
    W2 = np.asarray(W2, dtype=np.float32)
    b2 = np.asarray(b2, dtype=np.float32)

    T27 = _coeff_transform()
    W2p = np.ascontiguousarray(
        (W2.astype(np.float64) @ T27.T).astype(np.float32).reshape(MT, P, MOUT)
    )
    b2p_flat = (b2.astype(np.float64) @ T27.T).astype(np.float32)

    w1_r = W1.reshape(KT, P, HID)
    b1rep = np.ascontiguousarray(np.broadcast_to(b1, (SPC, HID)))
    b2bc = np.ascontiguousarray(
        np.broadcast_to(np.tile(b2p_flat, SPC), (P, SPC * MOUT))
    )
    eye2 = np.eye(SPC, dtype=np.float32)

    in_maps = []
    for core in range(N_CORES):
        xs = x[core * SPC : (core + 1) * SPC].reshape(-1)
        x_core = np.ascontiguousarray(xs.reshape(NTILES, P, T * C))
        h_core = hist[core * SPC : (core + 1) * SPC]  # [SPC, HIST]
        hp = np.ascontiguousarray(
            h_core.reshape(SPC, KT, P).transpose(2, 1, 0).reshape(P, KT * SPC)
        )
        in_maps.append(
            {
                "x_core": x_core,
                "h_packed": hp,
                "w1": w1_r,
                "b1_rep": b1rep,
                "eye2": eye2,
                "w2p": W2p,
                "b2bc": b2bc,
            }
        )
    return in_maps


def run(trace=False, **inputs):
    if "nc" not in _CACHE:
        _CACHE["nc"] = _build()
    nc = _CACHE["nc"]
    in_maps = _prep_inputs(**inputs)
    res = bass_utils.run_bass_kernel_spmd(
        nc, in_maps, core_ids=list(range(N_CORES)), trace=trace
    )
    outs = np.stack([r["y_core"] for r in res.results])  # [8, NTILES, P, T*C]
    y = outs.reshape(B, H, W, C).astype(np.float32)
    return y, res


def kernel(**inputs) -> np.ndarray:
    y, _ = run(trace=False, **inputs)
    return y


if __name__ == "__main__":
    rng = np.random.default_rng(0)
    ins = {
        "x": rng.random((B, H, W, C), dtype=np.float32),
        "histogram": rng.random((B, 3, 64, 64), dtype=np.float32),
        "W1": (rng.standard_normal((HIST, HID)) / np.sqrt(HIST)).astype(np.float32),
        "b1": np.zeros(HID, np.float32),
        "W2": (rng.standard_normal((HID, MOUT)) / np.sqrt(HID)).astype(np.float32),
        "b2": np.zeros(MOUT, np.float32),
    }
    y = kernel(**ins)
    print("out", y.shape, y.dtype, float(np.abs(y).max()))



# revision 3
# speedup vs baseline: 1.7969x; 1.7969x over previous
"""AWBNet (wo R2) Trainium2 kernel — v2.

Math (per sample b):
  m = reshape(relu(hist_flat @ W1 + b1) @ W2 + b2, [9, 3])
  feats(px) = [r, g, b, r^2, g^2, b^2, rg, rb, gb]
  y[px, c] = sum_k feats[px, k] * m[k, c]

Device strategy (8 cores, data parallel over batch, 2 samples/core):
  * Samples are split across SBUF partitions (sample 0 -> partitions 0..63,
    sample 1 -> 64..127), so one per-partition scalar vector carries the
    right m coefficients for every partition and ops span the full width.
  * Host packs x into fp16 channel planes (pure layout/dtype change), so
    the kernel never de-interleaves; W1 is streamed in fp16.
  * Tiny MLP on TensorE; m is broadcast per-partition by matmuls whose
    lhsT is a stride-0 broadcast column of featT (per 64-partition half).
  * Per-pixel combine y_c = sum_k m_kc B_k is split across engines:
      - 9 shared basis planes: r,g,b from DMA; squares on ACT; crosses on DVE
      - most (k,c) terms accumulate on the otherwise idle TensorE as
        diag(m_kc) @ B_k matmuls into PSUM (per-partition diagonals also
        encode the per-sample coefficients)
      - the rest form an SBUF fp16 partial on DVE (tensor_scalar products
        at 4x + adds) with some products on ACT; one extra identity matmul
        folds the partial into PSUM
      - ACT evicts PSUM -> fp16 SBUF, DMA writes planes out, host
        re-interleaves.
"""

import sys

import numpy as np

for _p in ("/opt/trn_rl_repo",):
    if _p not in sys.path:
        sys.path.insert(0, _p)

import concourse.bacc as bacc
import concourse.mybir as mybir
import concourse.tile as tile
from concourse import bass_utils

# ---- problem constants (hardcoded per contract) ----
N_CORES = 8
B, H, W, C = 16, 512, 512, 3
SPC = B // N_CORES  # samples per core = 2
PX_SAMPLE = H * W  # 262144
P = 128
PPS = P // SPC  # partitions per sample = 64
LANE_PX = PX_SAMPLE // PPS  # 4096 pixels per partition
NT = 2  # pixel tiles per core
TFD = LANE_PX // NT  # 2048 free-dim per tile
NCHUNK = TFD // 512  # 4 psum chunks of 512 per tile

HIST = 3 * 64 * 64  # 12288
HID = 256
MOUT = 27
KT = HIST // P  # 96 k-tiles
MT = HID // P  # 2 m-tiles
W1_CH = 8  # k-tiles per W1 DMA chunk

F16 = mybir.dt.float16
F32 = mybir.dt.float32
MULT = mybir.AluOpType.mult
ADD = mybir.AluOpType.add

# basis order: 0:r 1:g 2:b 3:r2 4:g2 5:b2 6:rg 7:rb 8:gb
# split of the 9 coefficients per channel across engines
PE_K = (0, 1, 2, 3, 4)  # accumulated on TensorE via diag matmuls
DVE_K = (5, 6)  # tensor_scalar product on DVE
ACT_K = (7, 8)  # product on ACT, added on DVE

_CACHE = {}


def _build():
    nc = bacc.Bacc(
        "TRN2", target_bir_lowering=False, debug=False, num_devices=N_CORES
    )

    xp_d = nc.dram_tensor("xp", [NT, P, C, TFD], F16, kind="ExternalInput")
    w1_d = nc.dram_tensor("w1pm", [P, KT, HID], F16, kind="ExternalInput")
    hp_d = nc.dram_tensor("h_packed", [P, KT * SPC], F16, kind="ExternalInput")
    b1_d = nc.dram_tensor("b1_rep", [SPC, HID], F32, kind="ExternalInput")
    w2_d = nc.dram_tensor("w2p", [MT, P, MOUT], F32, kind="ExternalInput")
    b2_d = nc.dram_tensor("b2bc", [P, MOUT], F32, kind="ExternalInput")
    eye2_d = nc.dram_tensor("eye2", [SPC, SPC], F32, kind="ExternalInput")
    eyeP_d = nc.dram_tensor("eyeP", [P, P], F16, kind="ExternalInput")
    y_d = nc.dram_tensor("y_planes", [C, NT, P, TFD], F16, kind="ExternalOutput")

    n_pe = len(PE_K)

    with tile.TileContext(nc) as tc:
        with (
            tc.tile_pool(name="mlp", bufs=1) as mlp_pool,
            tc.tile_pool(name="w1s", bufs=3) as w1_pool,
            tc.tile_pool(name="px", bufs=1) as px_pool,
            tc.tile_pool(name="tmp", bufs=2) as tmp_pool,
        ):
            # ---------------- setup DMAs (small, SWDGE) ----------------
            hp_sb = mlp_pool.tile([P, KT * SPC], F16, tag="hp", name="hp")
            nc.gpsimd.dma_start(out=hp_sb, in_=hp_d[:, :])
            b1_sb = mlp_pool.tile([SPC, HID], F32, tag="b1", name="b1")
            nc.gpsimd.dma_start(out=b1_sb, in_=b1_d[:, :])
            w2_sb = mlp_pool.tile([P, MT, MOUT], F32, tag="w2", name="w2")
            nc.gpsimd.dma_start(out=w2_sb, in_=w2_d.rearrange("m p n -> p m n"))
            b2_sb = mlp_pool.tile([P, MOUT], F32, tag="b2", name="b2")
            nc.gpsimd.dma_start(out=b2_sb, in_=b2_d[:, :])
            eye2_sb = mlp_pool.tile([SPC, SPC], F32, tag="eye2", name="eye2")
            nc.gpsimd.dma_start(out=eye2_sb, in_=eye2_d[:, :])
            eyeP_sb = mlp_pool.tile([P, P], F16, tag="eyeP", name="eyeP")
            nc.gpsimd.dma_start(out=eyeP_sb, in_=eyeP_d[:, :])

            # x planes (HWDGE via scalar queue)
            rgb = []
            for t in range(NT):
                r_t = px_pool.tile([P, C, TFD], F16, tag=f"rgb{t}", name=f"rgb{t}")
                nc.scalar.dma_start(out=r_t, in_=xp_d[t])
                rgb.append(r_t)

            # ---------------- MLP (TensorE) ----------------
            with tc.tile_pool(name="mlpps", bufs=1, space="PSUM") as mlp_psum:
                feat_ps = mlp_psum.tile([SPC, HID], F32, tag="featps", name="featps")
                for kc in range(KT // W1_CH):
                    w1_sb = w1_pool.tile([P, W1_CH, HID], F16, tag="w1c", name="w1c")
                    nc.sync.dma_start(
                        out=w1_sb, in_=w1_d[:, kc * W1_CH : (kc + 1) * W1_CH, :]
                    )
                    for kk in range(W1_CH):
                        k = kc * W1_CH + kk
                        nc.tensor.matmul(
                            feat_ps,
                            hp_sb[:, k * SPC : (k + 1) * SPC],
                            w1_sb[:, kk, :],
                            start=(k == 0),
                            stop=(k == KT - 1),
                        )

                # relu(feat + b1) on DVE
                feat_sb = mlp_pool.tile([SPC, HID], F32, tag="featsb", name="featsb")
                nc.vector.tensor_add(feat_sb, feat_ps, b1_sb)
                feat_r = mlp_pool.tile([SPC, HID], F32, tag="featr", name="featr")
                nc.vector.tensor_scalar(
                    feat_r, feat_sb, 0.0, None, mybir.AluOpType.max
                )

                # transpose feat [2, 256] -> featT tiles [128, 2] via PE
                featT_sb = []
                for mt in range(MT):
                    ft_ps = mlp_psum.tile(
                        [P, SPC], F32, tag=f"ftps{mt}", name=f"ftps{mt}"
                    )
                    nc.tensor.transpose(
                        ft_ps, feat_r[:, mt * P : (mt + 1) * P], eye2_sb
                    )
                    ft_sb = mlp_pool.tile(
                        [P, SPC], F32, tag=f"ftsb{mt}", name=f"ftsb{mt}"
                    )
                    nc.vector.tensor_copy(ft_sb, ft_ps)
                    featT_sb.append(ft_sb)

                # m-matmul with per-half broadcast: partitions 64s..64s+63 get
                # sample s's coefficients.
                ms_ps = mlp_psum.tile([P, MOUT], F32, tag="msps", name="msps")
                for s in range(SPC):
                    for mt in range(MT):
                        nc.tensor.matmul(
                            ms_ps[s * PPS : (s + 1) * PPS, :],
                            featT_sb[mt][:, s : s + 1].broadcast_to([P, PPS]),
                            w2_sb[:, mt, :],
                            start=(mt == 0),
                            stop=(mt == MT - 1),
                        )
                mscal = mlp_pool.tile([P, MOUT], F32, tag="mscal", name="mscal")
                nc.vector.tensor_add(mscal, ms_ps, b2_sb)

            def ms(k, c):
                j = 3 * k + c
                return mscal[:, j : j + 1]

            # ---------------- shared basis planes ----------------
            # squares on ACT, crosses on DVE (overlaps the W1 stream)
            sq = []
            cross = []
            for t in range(NT):
                sq_t = px_pool.tile([P, C, TFD], F16, tag=f"sq{t}", name=f"sq{t}")
                nc.scalar.square(sq_t, rgb[t])
                sq.append(sq_t)
                cr_t = px_pool.tile([P, C, TFD], F16, tag=f"cr{t}", name=f"cr{t}")
                nc.vector.tensor_mul(cr_t[:, 0, :], rgb[t][:, 0, :], rgb[t][:, 1, :])
                nc.vector.tensor_mul(cr_t[:, 1, :], rgb[t][:, 0, :], rgb[t][:, 2, :])
                nc.vector.tensor_mul(cr_t[:, 2, :], rgb[t][:, 1, :], rgb[t][:, 2, :])
                cross.append(cr_t)

            def basis(k, t):
                if k < 3:
                    return rgb[t][:, k, :]
                if k < 6:
                    return sq[t][:, k - 3, :]
                return cross[t][:, k - 6, :]

            # diag(m_kc) weights for the PE-accumulated terms
            diags = mlp_pool.tile([P, C, n_pe, P], F16, tag="diags", name="diags")
            for c in range(C):
                for i, k in enumerate(PE_K):
                    nc.vector.tensor_scalar(
                        diags[:, c, i, :], eyeP_sb, ms(k, c), None, MULT
                    )

            # ---------------- per-(tile, channel) combine ----------------
            with tc.tile_pool(name="pxps", bufs=2, space="PSUM") as px_psum:
                for t in range(NT):
                    for c in range(C):
                        # DVE/ACT partial: sum over DVE_K + ACT_K
                        part = tmp_pool.tile([P, TFD], F16, tag="part", name=f"pt{t}{c}")
                        k0 = DVE_K[0]
                        nc.vector.tensor_scalar(
                            part, basis(k0, t), ms(k0, c), None, MULT
                        )
                        for k in DVE_K[1:]:
                            u = tmp_pool.tile([P, TFD], F16, tag="u", name=f"u{t}{c}{k}")
                            nc.vector.tensor_scalar(u, basis(k, t), ms(k, c), None, MULT)
                            nc.vector.tensor_add(part, part, u)
                        for k in ACT_K:
                            v = tmp_pool.tile([P, TFD], F16, tag="v", name=f"v{t}{c}{k}")
                            nc.scalar.mul(v, basis(k, t), ms(k, c))
                            nc.vector.tensor_add(part, part, v)

                        # PE accumulation into PSUM
                        yc_ps = px_psum.tile([P, TFD], F32, tag="yc", name=f"yc{t}{c}")
                        for n in range(NCHUNK):
                            sl = slice(n * 512, (n + 1) * 512)
                            for i, k in enumerate(PE_K):
                                nc.tensor.matmul(
                                    yc_ps[:, sl],
                                    diags[:, c, i, :],
                                    basis(k, t)[:, sl],
                                    start=(i == 0),
                                    stop=False,
                                )
                            nc.tensor.matmul(
                                yc_ps[:, sl],
                                eyeP_sb,
                                part[:, sl],
                                start=False,
                                stop=True,
                            )

                        # evict PSUM -> fp16 SBUF on ACT, then DMA out
                        y_sb = tmp_pool.tile([P, TFD], F16, tag="ysb", name=f"y{t}{c}")
                        nc.scalar.copy(y_sb, yc_ps)
                        nc.sync.dma_start(out=y_d[c, t], in_=y_sb)

    nc.compile()
    return nc


def _prep_inputs(x, histogram, W1, b1, W2, b2):
    """Host-side sharding / layout packing (layout + dtype only; no data
    arithmetic)."""
    x = np.asarray(x, dtype=np.float32)
    hist = np.asarray(histogram, dtype=np.float32).reshape(B, HIST)
    W1 = np.asarray(W1, dtype=np.float32)
    b1 = np.asarray(b1, dtype=np.float32)
    W2 = np.asarray(W2, dtype=np.float32)
    b2 = np.asarray(b2, dtype=np.float32)

    # W1 partition-major fp16: w1pm[p, k, :] = W1[k*128 + p, :]
    w1pm = np.ascontiguousarray(
        W1.reshape(KT, P, HID).transpose(1, 0, 2)
    ).astype(np.float16)
    w2p = np.ascontiguousarray(W2.reshape(MT, P, MOUT))
    b1rep = np.ascontiguousarray(np.broadcast_to(b1, (SPC, HID)))
    b2bc = np.ascontiguousarray(np.broadcast_to(b2, (P, MOUT)))
    eye2 = np.eye(SPC, dtype=np.float32)
    eyeP = np.eye(P, dtype=np.float16)

    in_maps = []
    for core in range(N_CORES):
        # x planes: [NT, P, C, TFD] fp16, sample s on partitions 64s..64s+63
        xp = np.empty((NT, P, C, TFD), dtype=np.float16)
        for s in range(SPC):
            xs = x[core * SPC + s].reshape(PX_SAMPLE, C)
            # pixel index = p*LANE_PX + t*TFD + q
            v = xs.reshape(PPS, NT, TFD, C).transpose(1, 0, 3, 2)  # [t, p, c, q]
            xp[:, s * PPS : (s + 1) * PPS, :, :] = v.astype(np.float16)

        h_core = hist[core * SPC : (core + 1) * SPC]  # [SPC, HIST]
        hp = np.ascontiguousarray(
            h_core.reshape(SPC, KT, P).transpose(2, 1, 0).reshape(P, KT * SPC)
        ).astype(np.float16)
        in_maps.append(
            {
                "xp": xp,
                "w1pm": w1pm,
                "h_packed": hp,
                "b1_rep": b1rep,
                "w2p": w2p,
                "b2bc": b2bc,
                "eye2": eye2,
                "eyeP": eyeP,
            }
        )
    return in_maps


def _unpack_output(res):
    y = np.empty((B, H, W, C), dtype=np.float32)
    for core in range(N_CORES):
        planes = np.asarray(res.results[core]["y_planes"])  # [C, NT, P, TFD] f16
        for s in range(SPC):
            v = planes[:, :, s * PPS : (s + 1) * PPS, :]  # [C, NT, PPS, TFD]
            v = v.transpose(2, 1, 3, 0).reshape(PX_SAMPLE, C)  # px=(p,t,q)
            y[core * SPC + s] = v.reshape(H, W, C).astype(np.float32)
    return y


def run(trace=False, **inputs):
    if "nc" not in _CACHE:
        _CACHE["nc"] = _build()
    nc = _CACHE["nc"]
    in_maps = _prep_inputs(**inputs)
    res = bass_utils.run_bass_kernel_spmd(
        nc, in_maps, core_ids=list(range(N_CORES)), trace=trace
    )
    y = _unpack_output(res)
    return y, res


def kernel(**inputs) -> np.ndarray:
    y, _ = run(trace=False, **inputs)
    return y


if __name__ == "__main__":
    rng = np.random.default_rng(0)
    ins = {
        "x": rng.random((B, H, W, C), dtype=np.float32),
        "histogram": rng.random((B, 3, 64, 64), dtype=np.float32),
        "W1": (rng.standard_normal((HIST, HID)) / np.sqrt(HIST)).astype(np.float32),
        "b1": np.zeros(HID, np.float32),
        "W2": (rng.standard_normal((HID, MOUT)) / np.sqrt(HID)).astype(np.float32),
        "b2": np.zeros(MOUT, np.float32),
    }
    y = kernel(**ins)
    print("out", y.shape, y.dtype, float(np.abs(y).max()))


# revision 8
# speedup vs baseline: 1.9352x; 1.0769x over previous
"""AWBNet (wo R2) Trainium2 kernel — v2.

Math (per sample b):
  m = reshape(relu(hist_flat @ W1 + b1) @ W2 + b2, [9, 3])
  feats(px) = [r, g, b, r^2, g^2, b^2, rg, rb, gb]
  y[px, c] = sum_k feats[px, k] * m[k, c]

Device strategy (8 cores, data parallel over batch, 2 samples/core):
  * Samples are split across SBUF partitions (sample 0 -> partitions 0..63,
    sample 1 -> 64..127), so one per-partition scalar vector carries the
    right m coefficients for every partition and ops span the full width.
  * Host packs x into fp16 channel planes (pure layout/dtype change), so
    the kernel never de-interleaves; W1 is streamed in fp16.
  * Tiny MLP on TensorE; m is broadcast per-partition by matmuls whose
    lhsT is a stride-0 broadcast column of featT (per 64-partition half).
  * Per-pixel combine y_c = sum_k m_kc B_k is split across engines:
      - 9 shared basis planes: r,g,b from DMA; squares on ACT; crosses on DVE
      - most (k,c) terms accumulate on the otherwise idle TensorE as
        diag(m_kc) @ B_k matmuls into PSUM (per-partition diagonals also
        encode the per-sample coefficients)
      - the rest form an SBUF fp16 partial on DVE (tensor_scalar products
        at 4x + adds) with some products on ACT; one extra identity matmul
        folds the partial into PSUM
      - ACT evicts PSUM -> fp16 SBUF, DMA writes planes out, host
        re-interleaves.
"""

import sys

import numpy as np

for _p in ("/opt/trn_rl_repo",):
    if _p not in sys.path:
        sys.path.insert(0, _p)

import concourse.bacc as bacc
import concourse.mybir as mybir
import concourse.tile as tile
from concourse import bass_utils

# ---- problem constants (hardcoded per contract) ----
N_CORES = 8
B, H, W, C = 16, 512, 512, 3
SPC = B // N_CORES  # samples per core = 2
PX_SAMPLE = H * W  # 262144
P = 128
PPS = P // SPC  # partitions per sample = 64
LANE_PX = PX_SAMPLE // PPS  # 4096 pixels per partition
NT = 2  # pixel tiles per core
TFD = LANE_PX // NT  # 2048 free-dim per tile
NCHUNK = TFD // 512  # 4 psum chunks of 512 per tile

HIST = 3 * 64 * 64  # 12288
HID = 256
MOUT = 27
KT = HIST // P  # 96 k-tiles
MT = HID // P  # 2 m-tiles
W1_CH = 8  # k-tiles per W1 DMA chunk

F16 = mybir.dt.float16
F32 = mybir.dt.float32
MULT = mybir.AluOpType.mult
ADD = mybir.AluOpType.add

# basis order: 0:r 1:g 2:b 3:r2 4:g2 5:b2 6:rg 7:rb 8:gb
# split of the 9 coefficients per channel across engines
PE_K = (0, 1, 2, 3, 4)  # accumulated on TensorE via diag matmuls
DVE_K = (5, 6, 7)  # tensor_scalar product on DVE
ACT_K = (8,)  # product on ACT, added on DVE

_CACHE = {}


def _build():
    nc = bacc.Bacc(
        "TRN2", target_bir_lowering=False, debug=False, num_devices=N_CORES
    )

    xp_d = nc.dram_tensor("xp", [NT, P, C, TFD], F16, kind="ExternalInput")
    w1_d = nc.dram_tensor("w1pm", [P, KT, HID], F16, kind="ExternalInput")
    hp_d = nc.dram_tensor("h_packed", [P, KT * SPC], F16, kind="ExternalInput")
    b1_d = nc.dram_tensor("b1_rep", [SPC, HID], F32, kind="ExternalInput")
    w2_d = nc.dram_tensor("w2p", [MT, P, MOUT], F32, kind="ExternalInput")
    b2_d = nc.dram_tensor("b2bc", [P, MOUT], F32, kind="ExternalInput")
    eye2_d = nc.dram_tensor("eye2", [SPC, SPC], F32, kind="ExternalInput")
    eyeP_d = nc.dram_tensor("eyeP", [P, P], F16, kind="ExternalInput")
    y_d = nc.dram_tensor("y_planes", [C, NT, P, TFD], F16, kind="ExternalOutput")

    n_pe = len(PE_K)

    with tile.TileContext(nc) as tc:
        with (
            tc.tile_pool(name="mlp", bufs=1) as mlp_pool,
            tc.tile_pool(name="w1s", bufs=1) as w1_pool,
            tc.tile_pool(name="px", bufs=1) as px_pool,
            tc.tile_pool(name="tmp", bufs=2) as tmp_pool,
        ):
            # ---------------- input DMAs ----------------
            # hp + first W1 chunks first (critical path to m); W1 stream is
            # split across the sync + gpsimd queues, x planes ride scalar.
            hp_sb = mlp_pool.tile([P, KT * SPC], F16, tag="hp", name="hp")
            nc.gpsimd.dma_start(out=hp_sb, in_=hp_d[:, :])

            NCHW1 = KT // W1_CH  # 12 W1 chunks, all resident (48KB/partition)
            w1_sbs = []
            for kc in range(NCHW1):
                w1_sb = w1_pool.tile([P, W1_CH, HID], F16, tag=f"w1c{kc}", name=f"w1c{kc}")
                q = nc.sync if kc % 2 == 0 else nc.gpsimd
                q.dma_start(out=w1_sb, in_=w1_d[:, kc * W1_CH : (kc + 1) * W1_CH, :])
                w1_sbs.append(w1_sb)

            # x planes (HWDGE via scalar queue)
            rgb = []
            for t in range(NT):
                r_t = px_pool.tile([P, C, TFD], F16, tag=f"rgb{t}", name=f"rgb{t}")
                nc.scalar.dma_start(out=r_t, in_=xp_d[t])
                rgb.append(r_t)

            # small setup DMAs
            b1_sb = mlp_pool.tile([SPC, HID], F32, tag="b1", name="b1")
            nc.scalar.dma_start(out=b1_sb, in_=b1_d[:, :])
            w2_sb = mlp_pool.tile([P, MT, MOUT], F32, tag="w2", name="w2")
            nc.scalar.dma_start(out=w2_sb, in_=w2_d.rearrange("m p n -> p m n"))
            b2_sb = mlp_pool.tile([P, MOUT], F32, tag="b2", name="b2")
            nc.scalar.dma_start(out=b2_sb, in_=b2_d[:, :])
            eye2_sb = mlp_pool.tile([SPC, SPC], F32, tag="eye2", name="eye2")
            nc.scalar.dma_start(out=eye2_sb, in_=eye2_d[:, :])
            eyeP_sb = mlp_pool.tile([P, P], F16, tag="eyeP", name="eyeP")
            nc.scalar.dma_start(out=eyeP_sb, in_=eyeP_d[:, :])

            # ---------------- MLP (TensorE) ----------------
            with tc.tile_pool(name="mlpps", bufs=1, space="PSUM") as mlp_psum:
                feat_ps = mlp_psum.tile([SPC, HID], F32, tag="featps", name="featps")
                for kc in range(NCHW1):
                    w1_sb = w1_sbs[kc]
                    for kk in range(W1_CH):
                        k = kc * W1_CH + kk
                        nc.tensor.matmul(
                            feat_ps,
                            hp_sb[:, k * SPC : (k + 1) * SPC],
                            w1_sb[:, kk, :],
                            start=(k == 0),
                            stop=(k == KT - 1),
                        )

                # relu(feat + b1) on DVE
                feat_sb = mlp_pool.tile([SPC, HID], F32, tag="featsb", name="featsb")
                nc.vector.tensor_add(feat_sb, feat_ps, b1_sb)
                feat_r = mlp_pool.tile([SPC, HID], F32, tag="featr", name="featr")
                nc.vector.tensor_scalar(
                    feat_r, feat_sb, 0.0, None, mybir.AluOpType.max
                )

                # transpose feat [2, 256] -> featT tiles [128, 2] via PE
                featT_sb = []
                for mt in range(MT):
                    ft_ps = mlp_psum.tile(
                        [P, SPC], F32, tag=f"ftps{mt}", name=f"ftps{mt}"
                    )
                    nc.tensor.transpose(
                        ft_ps, feat_r[:, mt * P : (mt + 1) * P], eye2_sb
                    )
                    ft_sb = mlp_pool.tile(
                        [P, SPC], F32, tag=f"ftsb{mt}", name=f"ftsb{mt}"
                    )
                    nc.vector.tensor_copy(ft_sb, ft_ps)
                    featT_sb.append(ft_sb)

                # m-matmul with per-half broadcast: partitions 64s..64s+63 get
                # sample s's coefficients.
                ms_ps = mlp_psum.tile([P, MOUT], F32, tag="msps", name="msps")
                for s in range(SPC):
                    for mt in range(MT):
                        nc.tensor.matmul(
                            ms_ps[s * PPS : (s + 1) * PPS, :],
                            featT_sb[mt][:, s : s + 1].broadcast_to([P, PPS]),
                            w2_sb[:, mt, :],
                            start=(mt == 0),
                            stop=(mt == MT - 1),
                        )
                mscal = mlp_pool.tile([P, MOUT], F32, tag="mscal", name="mscal")
                nc.vector.tensor_add(mscal, ms_ps, b2_sb)

            def ms(k, c):
                j = 3 * k + c
                return mscal[:, j : j + 1]

            # ---------------- shared basis planes ----------------
            # squares on ACT, crosses on DVE (overlaps the W1 stream)
            sq = []
            cross = []
            for t in range(NT):
                sq_t = px_pool.tile([P, C, TFD], F16, tag=f"sq{t}", name=f"sq{t}")
                nc.scalar.square(sq_t, rgb[t])
                sq.append(sq_t)
                cr_t = px_pool.tile([P, C, TFD], F16, tag=f"cr{t}", name=f"cr{t}")
                nc.vector.tensor_mul(cr_t[:, 0, :], rgb[t][:, 0, :], rgb[t][:, 1, :])
                nc.vector.tensor_mul(cr_t[:, 1, :], rgb[t][:, 0, :], rgb[t][:, 2, :])
                nc.vector.tensor_mul(cr_t[:, 2, :], rgb[t][:, 1, :], rgb[t][:, 2, :])
                cross.append(cr_t)

            def basis(k, t):
                if k < 3:
                    return rgb[t][:, k, :]
                if k < 6:
                    return sq[t][:, k - 3, :]
                return cross[t][:, k - 6, :]

            # diag(m_kc) weights for the PE-accumulated terms
            diags = mlp_pool.tile([P, C, n_pe, P], F16, tag="diags", name="diags")
            for c in range(C):
                for i, k in enumerate(PE_K):
                    nc.vector.tensor_scalar(
                        diags[:, c, i, :], eyeP_sb, ms(k, c), None, MULT
                    )

            # ---------------- per-(tile, channel) combine ----------------
            with tc.tile_pool(name="pxps", bufs=2, space="PSUM") as px_psum:
                for t in range(NT):
                    for c in range(C):
                        # DVE/ACT partial: sum over DVE_K + ACT_K
                        part = tmp_pool.tile([P, TFD], F16, tag="part", name=f"pt{t}{c}")
                        k0 = DVE_K[0]
                        nc.vector.tensor_scalar(
                            part, basis(k0, t), ms(k0, c), None, MULT
                        )
                        for k in DVE_K[1:]:
                            u = tmp_pool.tile([P, TFD], F16, tag="u", name=f"u{t}{c}{k}")
                            nc.vector.tensor_scalar(u, basis(k, t), ms(k, c), None, MULT)
                            nc.vector.tensor_add(part, part, u)
                        for k in ACT_K:
                            v = tmp_pool.tile([P, TFD], F16, tag="v", name=f"v{t}{c}{k}")
                            nc.scalar.mul(v, basis(k, t), ms(k, c))
                            nc.vector.tensor_add(part, part, v)

                        # PE accumulation into PSUM (k outer: consecutive
                        # matmuls share the stationary diag -> LDW amortized)
                        yc_ps = px_psum.tile([P, TFD], F32, tag="yc", name=f"yc{t}{c}")
                        for i, k in enumerate(PE_K):
                            for n in range(NCHUNK):
                                sl = slice(n * 512, (n + 1) * 512)
                                nc.tensor.matmul(
                                    yc_ps[:, sl],
                                    diags[:, c, i, :],
                                    basis(k, t)[:, sl],
                                    start=(i == 0),
                                    stop=False,
                                )
                        for n in range(NCHUNK):
                            sl = slice(n * 512, (n + 1) * 512)
                            nc.tensor.matmul(
                                yc_ps[:, sl],
                                eyeP_sb,
                                part[:, sl],
                                start=False,
                                stop=True,
                            )

                        # evict PSUM -> fp16 SBUF on ACT, then DMA out
                        y_sb = tmp_pool.tile([P, TFD], F16, tag="ysb", name=f"y{t}{c}")
                        nc.scalar.copy(y_sb, yc_ps)
                        nc.sync.dma_start(out=y_d[c, t], in_=y_sb)

    nc.compile()
    return nc


def _prep_inputs(x, histogram, W1, b1, W2, b2):
    """Host-side sharding / layout packing (layout + dtype only; no data
    arithmetic)."""
    x = np.asarray(x, dtype=np.float32)
    hist = np.asarray(histogram, dtype=np.float32).reshape(B, HIST)
    W1 = np.asarray(W1, dtype=np.float32)
    b1 = np.asarray(b1, dtype=np.float32)
    W2 = np.asarray(W2, dtype=np.float32)
    b2 = np.asarray(b2, dtype=np.float32)

    # W1 partition-major fp16: w1pm[p, k, :] = W1[k*128 + p, :]
    w1pm = np.ascontiguousarray(
        W1.reshape(KT, P, HID).transpose(1, 0, 2)
    ).astype(np.float16)
    w2p = np.ascontiguousarray(W2.reshape(MT, P, MOUT))
    b1rep = np.ascontiguousarray(np.broadcast_to(b1, (SPC, HID)))
    b2bc = np.ascontiguousarray(np.broadcast_to(b2, (P, MOUT)))
    eye2 = np.eye(SPC, dtype=np.float32)
    eyeP = np.eye(P, dtype=np.float16)

    in_maps = []
    for core in range(N_CORES):
        # x planes: [NT, P, C, TFD] fp16, sample s on partitions 64s..64s+63
        xp = np.empty((NT, P, C, TFD), dtype=np.float16)
        for s in range(SPC):
            xs = x[core * SPC + s].reshape(PX_SAMPLE, C)
            # pixel index = p*LANE_PX + t*TFD + q
            v = xs.reshape(PPS, NT, TFD, C).transpose(1, 0, 3, 2)  # [t, p, c, q]
            xp[:, s * PPS : (s + 1) * PPS, :, :] = v.astype(np.float16)

        h_core = hist[core * SPC : (core + 1) * SPC]  # [SPC, HIST]
        hp = np.ascontiguousarray(
            h_core.reshape(SPC, KT, P).transpose(2, 1, 0).reshape(P, KT * SPC)
        ).astype(np.float16)
        in_maps.append(
            {
                "xp": xp,
                "w1pm": w1pm,
                "h_packed": hp,
                "b1_rep": b1rep,
                "w2p": w2p,
                "b2bc": b2bc,
                "eye2": eye2,
                "eyeP": eyeP,
            }
        )
    return in_maps


def _unpack_output(res):
    y = np.empty((B, H, W, C), dtype=np.float32)
    for core in range(N_CORES):
        planes = np.asarray(res.results[core]["y_planes"])  # [C, NT, P, TFD] f16
        for s in range(SPC):
            v = planes[:, :, s * PPS : (s + 1) * PPS, :]  # [C, NT, PPS, TFD]
            v = v.transpose(2, 1, 3, 0).reshape(PX_SAMPLE, C)  # px=(p,t,q)
            y[core * SPC + s] = v.reshape(H, W, C).astype(np.float32)
    return y


def run(trace=False, **inputs):
    if "nc" not in _CACHE:
        _CACHE["nc"] = _build()
    nc = _CACHE["nc"]
    in_maps = _prep_inputs(**inputs)
    res = bass_utils.run_bass_kernel_spmd(
        nc, in_maps, core_ids=list(range(N_CORES)), trace=trace
    )
    y = _unpack_output(res)
    return y, res


def kernel(**inputs) -> np.ndarray:
    y, _ = run(trace=False, **inputs)
    return y


if __name__ == "__main__":
    rng = np.random.default_rng(0)
    ins = {
        "x": rng.random((B, H, W, C), dtype=np.float32),
        "histogram": rng.random((B, 3, 64, 64), dtype=np.float32),
        "W1": (rng.standard_normal((HIST, HID)) / np.sqrt(HIST)).astype(np.float32),
        "b1": np.zeros(HID, np.float32),
        "W2": (rng.standard_normal((HID, MOUT)) / np.sqrt(HID)).astype(np.float32),
        "b2": np.zeros(MOUT, np.float32),
    }
    y = kernel(**ins)
    print("out", y.shape, y.dtype, float(np.abs(y).max()))
